# revision 15
# baseline (speedup 1.0000x reference)
"""Causal self-attention (B=4, T=2048, C=768, H=12) on 8 TRN2 NeuronCores.

Sharding: core c -> (batch b = c//2, head-group g = c%2 of 6 heads).
Each core computes the qkv projection for its 6 heads, causal attention, and a
row-parallel partial of the output projection. The host sums the two partials
per batch and adds a constant correction (bout + bv @ Wout: the v-bias
commutes through softmax rows, and the k-bias cancels inside softmax).

Device layout choices (no on-device transposes needed):
  xt    = x[b].T                        [C, T]   (host pre-transposed)
  qT,kT = per-head-dim-on-partitions    [384, T] (head pair per 128-tile)
  v     = x @ Wv                        [T, 384] (natural)
  S_T[t_k, t_q] = kT-slice.T @ qT-slice; causal mask added in PSUM by an
  identity-matmul against a step matrix; exp on ScalarE (scores are O(1), so
  no max-subtraction); the softmax denominator rides as an extra M=1 matmul
  concurrent with P@V (disjoint PE column groups); normalization is a K=1
  broadcast matmul pair + one copy + per-half vector multiplies.

All matmul operands are float32r (full-rate PE at N>=256; fp32 bits, reduced
multiply precision) with fp32 PSUM accumulation.
"""

import numpy as np

import concourse.bacc as bacc
import concourse.bass as bass
import concourse.mybir as mybir
import concourse.tile as tile
from concourse.bass_utils import run_bass_kernel_spmd

B, T, C = 4, 2048, 768
H, D = 12, 64
HL = 6              # local heads per core
TL = HL * D         # 384 local channels
CT = C // 128       # 6 contraction tiles
JT = TL // 128      # 3 local j-tiles
QS = T // 512       # 4 query slices
KB = T // 128       # 16 key blocks
NEGBIG = -1.0e30

F32 = mybir.dt.float32
F32R = mybir.dt.float32r
ID = mybir.ActivationFunctionType.Identity
EXP = mybir.ActivationFunctionType.Exp


def _mm(nc, out, lhsT, rhs, start, stop, skip_check=False):
    nc.tensor.matmul(out, lhsT, rhs, start=start, stop=stop,
                     skip_group_check=skip_check)


def build_nc():
    nc = bacc.Bacc("TRN2", target_bir_lowering=False, debug=False, num_devices=8)

    xt_d = nc.dram_tensor("xt", [C, T], F32R, kind="ExternalInput")
    wqkv_d = nc.dram_tensor("wqkv", [C, 3 * TL], F32R, kind="ExternalInput")
    bq_d = nc.dram_tensor("bq", [128, JT], F32, kind="ExternalInput")
    wout_d = nc.dram_tensor("wout", [TL, C], F32R, kind="ExternalInput")
    mask_d = nc.dram_tensor("maskstep", [128, 512], F32R, kind="ExternalInput")
    ident_d = nc.dram_tensor("ident", [128, 128], F32R, kind="ExternalInput")
    ones_d = nc.dram_tensor("ones", [128, 128], F32R, kind="ExternalInput")
    vones_d = nc.dram_tensor("vones", [128, KB * HL], F32R, kind="ExternalInput")
    out_d = nc.dram_tensor("out", [T, C], F32, kind="ExternalOutput")

    with (
        tile.TileContext(nc) as tc,
        nc.allow_low_precision(reason="float32r matmul inputs"),
        tc.tile_pool(name="persist", bufs=1) as persist,
    ):
        # ---- persistent SBUF tensors ----
        wqkv_sb = persist.tile([128, CT, 3 * TL], F32R)
        wout_sb = persist.tile([128, JT, C], F32R)
        bq_sb = persist.tile([128, JT], F32)
        mask_sb = persist.tile([128, 512], F32R)
        ident_sb = persist.tile([128, 128], F32R)
        ones_sb = persist.tile([128, 128], F32R)
        qt_sb = persist.tile([128, JT, T], F32R)
        kt_sb = persist.tile([128, JT, T], F32R)
        # per head l: cols [l*65, l*65+64) = V dims, col l*65+64 = ones
        v_sb = persist.tile([128, KB, HL * 65], F32R)
        attn_sb = persist.tile([128, JT, T], F32R)

        nc.sync.dma_start(
            out=wqkv_sb[:], in_=wqkv_d.ap().rearrange("(i p) n -> p i n", p=128)
        )
        nc.sync.dma_start(
            out=wout_sb[:], in_=wout_d.ap().rearrange("(i p) n -> p i n", p=128)
        )
        nc.sync.dma_start(out=bq_sb[:], in_=bq_d.ap())
        nc.sync.dma_start(out=mask_sb[:], in_=mask_d.ap())
        nc.sync.dma_start(out=ident_sb[:], in_=ident_d.ap())
        nc.sync.dma_start(out=ones_sb[:], in_=ones_d.ap())
        nc.sync.dma_start(
            out=v_sb[:].rearrange("p m (l c) -> p m l c", c=65)[:, :, :, 64],
            in_=vones_d.ap().rearrange("p (m l) -> p m l", l=HL),
        )

        # ---- phase 1: qkv projection ----
        with (
            tc.tile_pool(name="xchunk", bufs=2) as xchunk,
            tc.tile_pool(name="qk_ps", bufs=3, space=bass.MemorySpace.PSUM) as qk_ps,
            tc.tile_pool(name="v_ps", bufs=2, space=bass.MemorySpace.PSUM) as v_ps,
        ):
            for ts in range(QS):
                xtc = xchunk.tile([128, CT, 512], F32R, tag="xtc")
                nc.sync.dma_start(
                    out=xtc[:],
                    in_=xt_d.ap()[:, ts * 512 : (ts + 1) * 512].rearrange(
                        "(i p) t -> p i t", p=128
                    ),
                )
                tcols = slice(ts * 512, (ts + 1) * 512)
                for jt in range(JT):
                    jcols = slice(jt * 128, (jt + 1) * 128)
                    qp = qk_ps.tile([128, 512], F32, tag="qp")
                    for i in range(CT):
                        _mm(nc, qp[:], wqkv_sb[:, i, jcols], xtc[:, i, :],
                            start=(i == 0), stop=(i == CT - 1))
                    nc.scalar.activation(
                        qt_sb[:, jt, tcols], qp[:], ID,
                        bias=bq_sb[:, jt : jt + 1], scale=1.0,
                    )
                    kp = qk_ps.tile([128, 512], F32, tag="qp")
                    kcols = slice(TL + jt * 128, TL + (jt + 1) * 128)
                    for i in range(CT):
                        _mm(nc, kp[:], wqkv_sb[:, i, kcols], xtc[:, i, :],
                            start=(i == 0), stop=(i == CT - 1))
                    nc.vector.tensor_copy(kt_sb[:, jt, tcols], kp[:])
                for mm in range(4):
                    m = 4 * ts + mm
                    vp = v_ps.tile([128, TL], F32, tag="vp")
                    for i in range(CT):
                        _mm(nc, vp[:], xtc[:, i, mm * 128 : (mm + 1) * 128],
                            wqkv_sb[:, i, 2 * TL : 3 * TL],
                            start=(i == 0), stop=(i == CT - 1))
                    nc.vector.tensor_copy(
                        v_sb[:, m, :].rearrange("p (l c) -> p l c", c=65)[:, :, 0:64],
                        vp[:].rearrange("p (l c) -> p l c", c=64),
                    )

        # ---- phase 2: attention, head pairs (2*jt, 2*jt+1) ----
        with (
            tc.tile_pool(name="es2", bufs=3) as es2,
            tc.tile_pool(name="rc2", bufs=2) as rc2,
            tc.tile_pool(name="bcs2", bufs=2) as bcs2,
            tc.tile_pool(name="atmp2", bufs=2) as atmp2,
            tc.tile_pool(name="s_ps", bufs=2, space=bass.MemorySpace.PSUM) as s_ps,
            tc.tile_pool(name="pv_ps", bufs=1, space=bass.MemorySpace.PSUM) as pv_ps,
            tc.tile_pool(name="bc_ps", bufs=1, space=bass.MemorySpace.PSUM) as bc_ps,
        ):
            for jt in range(JT):
                for qb in range(QS):
                    # full k-blocks, then the 4 diagonal blocks (restricted cols)
                    blocks = [(kb, 0) for kb in range(4 * qb)] + [
                        (4 * qb + m, 512 - 128 * m) for m in range(4)
                    ]
                    nblk = len(blocks)
                    # rows 0..63: unnormalized P@V (head dims), row 64: sums
                    pv = [
                        pv_ps.tile([128, 512], F32, tag=f"pv{half}", name=f"pv{half}")
                        for half in (0, 1)
                    ]

                    def emit_qk(bi, jt=jt, qb=qb, blocks=blocks):
                        kb, diag = blocks[bi]
                        ncols = diag if diag else 512
                        q0 = (qb + 1) * 512 - ncols
                        sps = []
                        for half in (0, 1):
                            hs = slice(half * 64, half * 64 + 64)
                            sp = s_ps.tile([128, 512], F32, tag=f"s{half}",
                                           name=f"s{half}")
                            _mm(nc, sp[:, :ncols],
                                kt_sb[hs, jt, kb * 128 : (kb + 1) * 128],
                                qt_sb[hs, jt, q0 : (qb + 1) * 512],
                                start=True, stop=(diag == 0))
                            if diag:
                                _mm(nc, sp[:, :ncols], ident_sb[:],
                                    mask_sb[:, :ncols], start=False, stop=True)
                            sps.append((sp, ncols))
                        return sps

                    def emit_exp_pv(bi, sps, jt=jt, blocks=blocks, nblk=nblk, pv=pv):
                        kb, _ = blocks[bi]
                        for half in (0, 1):
                            sp, ncols = sps[half]
                            c0 = 512 - ncols
                            es = es2.tile([128, 512], F32R, tag=f"es{half}",
                                          name=f"es{half}")
                            nc.scalar.activation(
                                es[:, :ncols], sp[:, :ncols], EXP, scale=0.125
                            )
                            l = 2 * jt + half
                            _mm(nc, pv[half][0:65, c0:512],
                                v_sb[:, kb, l * 65 : (l + 1) * 65], es[:, :ncols],
                                start=(bi == 0), stop=(bi == nblk - 1))

                    LOOKAHEAD = 1
                    stage = []
                    for bi in range(nblk):
                        stage.append((bi, emit_qk(bi)))
                        if len(stage) > LOOKAHEAD:
                            pbi, psps = stage.pop(0)
                            emit_exp_pv(pbi, psps)
                    for pbi, psps in stage:
                        emit_exp_pv(pbi, psps)

                    # normalize + write attn_sb
                    qcols = slice(qb * 512, (qb + 1) * 512)
                    for half in (0, 1):
                        rc = rc2.tile([128, 512], F32R, tag=f"rc{half}",
                                      name=f"rc{half}")
                        nc.vector.reciprocal(rc[64:65, :], pv[half][64:65, :])
                        bc = bc_ps.tile([64, 512], F32, tag=f"bc{half}",
                                        name=f"bc{half}")
                        _mm(nc, bc[0:64, :], ones_sb[64:65, 0:64], rc[64:65, :],
                            start=True, stop=True)
                        bcs = bcs2.tile([64, 512], F32, tag=f"bcs{half}",
                                        name=f"bcs{half}")
                        nc.vector.tensor_copy(bcs[:], bc[0:64, :])
                        if half == 0:
                            nc.vector.tensor_mul(
                                attn_sb[0:64, jt, qcols], pv[half][0:64, :], bcs[:]
                            )
                        else:
                            at = atmp2.tile([64, 512], F32R, tag="at")
                            nc.vector.tensor_mul(at[:], pv[half][0:64, :], bcs[:])
                            nc.sync.dma_start(
                                out=attn_sb[64:128, jt, qcols], in_=at[:]
                            )

        # ---- phase 3: output projection (row-parallel partial) ----
        with (
            tc.tile_pool(name="op_ps", bufs=3, space=bass.MemorySpace.PSUM) as op_ps,
            tc.tile_pool(name="outsb", bufs=3) as outsb,
        ):
            for m in range(KB):
                op = op_ps.tile([128, C], F32, tag="op")
                mcols = slice(m * 128, (m + 1) * 128)
                for k in range(JT):
                    _mm(nc, op[:, 0:512], attn_sb[:, k, mcols],
                        wout_sb[:, k, 0:512], start=(k == 0), stop=(k == JT - 1))
                for k in range(JT):
                    _mm(nc, op[:, 512:C], attn_sb[:, k, mcols],
                        wout_sb[:, k, 512:C], start=(k == 0), stop=(k == JT - 1))
                osb = outsb.tile([128, C], F32, tag="osb")
                nc.vector.tensor_copy(osb[:], op[:])
                nc.sync.dma_start(out=out_d.ap()[mcols, :], in_=osb[:])

    nc.compile()
    return nc


_NC_CACHE = []


def _get_nc():
    if not _NC_CACHE:
        _NC_CACHE.append(build_nc())
    return _NC_CACHE[0]


def make_in_maps(x, Wqkv, bqkv, Wout):
    x = np.asarray(x, np.float32)
    Wqkv = np.asarray(Wqkv, np.float32)
    bqkv = np.asarray(bqkv, np.float32)
    Wout = np.asarray(Wout, np.float32)

    maskstep = np.zeros((128, 512), np.float32)
    for k in range(128):
        maskstep[k, :k] = NEGBIG
    ident = np.eye(128, dtype=np.float32)
    ones = np.ones((128, 128), np.float32)
    vones = np.ones((128, KB * HL), np.float32)

    in_maps = []
    for c in range(8):
        b, g = c // 2, c % 2
        cols = slice(g * TL, (g + 1) * TL)
        wq = Wqkv[:, 0:C][:, cols]
        wk = Wqkv[:, C : 2 * C][:, cols]
        wv = Wqkv[:, 2 * C : 3 * C][:, cols]
        bq = np.ascontiguousarray(bqkv[0:C][cols].reshape(JT, 128).T)
        in_maps.append(
            {
                "xt": np.ascontiguousarray(x[b].T),
                "wqkv": np.ascontiguousarray(np.concatenate([wq, wk, wv], axis=1)),
                "bq": bq,
                "wout": np.ascontiguousarray(Wout[cols, :]),
                "maskstep": maskstep,
                "ident": ident,
                "ones": ones,
                "vones": vones,
            }
        )
    return in_maps


def kernel(x, Wqkv, bqkv, Wout, bout):
    nc = _get_nc()
    in_maps = make_in_maps(x, Wqkv, bqkv, Wout)
    res = run_bass_kernel_spmd(nc, in_maps, core_ids=list(range(8)))

    bv = np.asarray(bqkv, np.float64)[2 * C : 3 * C]
    corr = bv @ np.asarray(Wout, np.float64) + np.asarray(bout, np.float64)
    out = np.empty((B, T, C), np.float32)
    for b in range(B):
        out[b] = (
            res.results[2 * b]["out"].astype(np.float64)
            + res.results[2 * b + 1]["out"]
            + corr
        ).astype(np.float32)
    return out
